# revision 6
# baseline (speedup 1.0000x reference)
"""Trainium2 Bass kernel for the DLGN kernel-machine problem.

Reference computation (fp32):
    ig = inp @ gating[0]; dg = data @ gating[0]
    K  = sig(B*ig) @ sig(B*dg).T
    for l in 1..3:
        ig = ig @ gating[l]; dg = dg @ gating[l]
        K *= (sig(B*ig) @ sig(B*dg).T) / 512
    out = K @ alphas                      # [n_inp]

Shapes: inp [4096, 512], data [8192, 512], gating [4, 512, 512],
alphas [8192]; out [4096] fp32.

Strategy (8 NeuronCores):
  - 2D shard: inp rows into R=4 groups of 1024 (replicated over C), data rows
    into C=2 groups of 4096 (replicated over R). core = r*C + c. Each core
    computes y_partial[r-block] = K_block @ alphas[c-block]; host sums the C
    partials and concatenates the R blocks. No on-device collectives.
  - All matmuls run as float32r (TF32-like, ~1.3e-4 rel err, full PE rate at
    N>=256) with the contraction dim on SBUF partitions. The host feeds the
    activations pre-transposed ([512, n]) so the gate chain
    igT_l = W_l^T-contract(igT_{l-1}) stays in transposed layout with zero
    on-device transposes.
  - Per core: phase A computes the i-side gate chain for all 4 layers
    (sig_i resident, 64KB/partition). Then 8 d-stripes of 512: d-side gate
    chain for the stripe, alphas folded into sig_d layer 3, then the K-product
    matmuls with the running elementwise product and the final row-sum reduce
    fused on the vector engine (tensor_tensor_reduce accumulates y directly).
"""

import numpy as np

import concourse.tile as tile
from concourse import bacc, mybir
from concourse.bass_utils import run_bass_kernel_spmd

BETA = 4.0
WIDTH = 512
DEPTH = 4
DIM = 512
N_I = 4096
N_D = 8192
R, C = 4, 2
NI_SH = N_I // R  # 1024
ND_SH = N_D // C  # 4096
D_STRIPE = 512
N_STRIPES = ND_SH // D_STRIPE  # 8
I_CHUNKS = NI_SH // 128  # 8
KC = DIM // 128  # 4 contraction chunks

F32 = mybir.dt.float32
F32R = mybir.dt.float32r
BF16 = mybir.dt.bfloat16
AFT = mybir.ActivationFunctionType
MULT = mybir.AluOpType.mult
ADD = mybir.AluOpType.add

_NC = None


def _build(repeat=1, sig_bf16=False):
    nc = bacc.Bacc("TRN2", target_bir_lowering=False, debug=False, num_devices=8)

    inpT_d = nc.dram_tensor("inpT", [DIM, NI_SH], F32R, kind="ExternalInput")
    dataT_d = nc.dram_tensor("dataT", [DIM, ND_SH], F32R, kind="ExternalInput")
    gating_d = nc.dram_tensor("gating", [DEPTH, DIM, DIM], F32R, kind="ExternalInput")
    alphas_d = nc.dram_tensor("alphas_b", [128, ND_SH], F32, kind="ExternalInput")
    y_d = nc.dram_tensor("y", [128, I_CHUNKS], F32, kind="ExternalOutput")

    SIG_DT = BF16 if sig_bf16 else F32R
    from contextlib import nullcontext

    with tile.TileContext(nc) as tc:
        with (
            tc.tile_pool(name="w", bufs=1) as wpool,
            tc.tile_pool(name="sigi", bufs=1) as sigi_pool,
            tc.tile_pool(name="yp", bufs=1) as ypool,
            tc.tile_pool(name="gpsum", bufs=2, space="PSUM") as gpsum,
            tc.tile_pool(name="kpsum", bufs=2, space="PSUM") as kpsum,
            tc.For_i(0, repeat, 1) if repeat > 1 else nullcontext(),
        ):
            W = wpool.tile([128, DEPTH, KC, DIM], F32R)
            for l in range(DEPTH):
                nc.sync.dma_start(
                    W[:, l],
                    gating_d.ap()[l].rearrange("(k p) n -> p k n", p=128),
                )

            sig_i = sigi_pool.tile([128, DEPTH, KC, NI_SH], SIG_DT)
            y_acc = ypool.tile([128, I_CHUNKS], F32)
            nc.gpsimd.memset(y_acc[:], 0.0)

            # ---- Phase A: i-side gate chain, all layers ----
            with tc.tile_pool(name="ig", bufs=2) as igpool:
                prev = igpool.tile([128, KC, NI_SH], F32R, tag="ig")
                inpT_r = inpT_d.ap().rearrange("(k p) n -> p k n", p=128)
                for k in range(KC):
                    nc.sync.dma_start(prev[:, k], inpT_r[:, k])
                for l in range(DEPTH):
                    nxt = (
                        igpool.tile([128, KC, NI_SH], F32R, tag="ig", name=f"ig{l}")
                        if l < DEPTH - 1
                        else None
                    )
                    for m in range(KC):
                        for nb in range(NI_SH // 512):
                            sl = slice(nb * 512, (nb + 1) * 512)
                            ps = gpsum.tile([128, 512], F32, tag="gps")
                            for k in range(KC):
                                nc.tensor.matmul(
                                    ps[:],
                                    W[:, l, k, m * 128 : (m + 1) * 128],
                                    prev[:, k, sl],
                                    start=(k == 0),
                                    stop=(k == KC - 1),
                                )
                            nc.scalar.activation(
                                sig_i[:, l, m, sl], ps[:], AFT.Sigmoid, scale=BETA
                            )
                            if nxt is not None:
                                nc.vector.tensor_copy(nxt[:, m, sl], ps[:])
                    prev = nxt

            # ---- Phase B: d-stripes ----
            with (
                tc.tile_pool(name="dat", bufs=2) as datpool,
                tc.tile_pool(name="dg", bufs=2) as dgpool,
                tc.tile_pool(name="sigd", bufs=2 if sig_bf16 else 1) as sigd_pool,
                tc.tile_pool(name="alp", bufs=2) as alpool,
                tc.tile_pool(name="kblk", bufs=2) as kpool,
                tc.tile_pool(name="scr", bufs=2) as scrpool,
            ):
                for s in range(N_STRIPES):
                    ssl = slice(s * D_STRIPE, (s + 1) * D_STRIPE)
                    dat = datpool.tile([128, KC, D_STRIPE], F32R, tag="dat")
                    nc.sync.dma_start(
                        dat[:],
                        dataT_d.ap()[:, ssl].rearrange("(k p) n -> p k n", p=128),
                    )
                    alp = alpool.tile([128, D_STRIPE], F32, tag="alp")
                    nc.sync.dma_start(alp[:], alphas_d.ap()[:, ssl])

                    sig_d = sigd_pool.tile([128, DEPTH, KC, D_STRIPE], SIG_DT)

                    # d-side gate chain for this stripe
                    prev = dat
                    for l in range(DEPTH):
                        nxt = (
                            dgpool.tile(
                                [128, KC, D_STRIPE], F32R, tag="dg", name=f"dg{s}_{l}"
                            )
                            if l < DEPTH - 1
                            else None
                        )
                        for m in range(KC):
                            ps = gpsum.tile([128, 512], F32, tag="gps")
                            for k in range(KC):
                                nc.tensor.matmul(
                                    ps[:],
                                    W[:, l, k, m * 128 : (m + 1) * 128],
                                    prev[:, k, :],
                                    start=(k == 0),
                                    stop=(k == KC - 1),
                                )
                            nc.scalar.activation(
                                sig_d[:, l, m, :], ps[:], AFT.Sigmoid, scale=BETA
                            )
                            if nxt is not None:
                                nc.vector.tensor_copy(nxt[:, m, :], ps[:])
                            else:
                                # layer 3: fold alphas into sig_d
                                nc.vector.tensor_mul(
                                    sig_d[:, 3, m, :], sig_d[:, 3, m, :], alp[:]
                                )
                        prev = nxt

                    # K-product matmuls + running product + y reduce
                    for ic in range(I_CHUNKS):
                        isl = slice(ic * 128, (ic + 1) * 128)
                        kblk = kpool.tile([128, D_STRIPE], F32, tag="kblk")
                        for l in range(DEPTH):
                            ps = kpsum.tile([128, 512], F32, tag="kps")
                            for k in range(KC):
                                nc.tensor.matmul(
                                    ps[:],
                                    sig_i[:, l, k, isl],
                                    sig_d[:, l, k, :],
                                    start=(k == 0),
                                    stop=(k == KC - 1),
                                )
                            if l == 0:
                                nc.vector.tensor_copy(kblk[:], ps[:])
                            elif l < DEPTH - 1:
                                # kblk = (ps * 1/512) * kblk
                                nc.vector.scalar_tensor_tensor(
                                    kblk[:], ps[:], 1.0 / WIDTH, kblk[:], MULT, MULT
                                )
                            else:
                                # y[:, ic] += sum_d (ps * 1/512) * kblk
                                part = scrpool.tile([128, 1], F32, tag="part")
                                nc.vector.scalar_tensor_tensor(
                                    kblk[:],
                                    ps[:],
                                    1.0 / WIDTH,
                                    kblk[:],
                                    MULT,
                                    MULT,
                                    accum_out=part[:, 0:1],
                                )
                                nc.vector.tensor_add(
                                    y_acc[:, ic : ic + 1],
                                    y_acc[:, ic : ic + 1],
                                    part[:, 0:1],
                                )

            nc.sync.dma_start(y_d.ap(), y_acc[:])

    nc.compile()
    return nc


def _get_nc():
    global _NC
    if _NC is None:
        _NC = _build()
    return _NC


def kernel(inp, data, gating, alphas):
    inp = np.ascontiguousarray(np.asarray(inp, dtype=np.float32))
    data = np.ascontiguousarray(np.asarray(data, dtype=np.float32))
    gating = np.ascontiguousarray(np.asarray(gating, dtype=np.float32))
    alphas = np.ascontiguousarray(np.asarray(alphas, dtype=np.float32))

    nc = _get_nc()

    in_maps = []
    for r in range(R):
        inpT = np.ascontiguousarray(inp[r * NI_SH : (r + 1) * NI_SH].T)
        for c in range(C):
            dataT = np.ascontiguousarray(data[c * ND_SH : (c + 1) * ND_SH].T)
            al = np.ascontiguousarray(
                np.broadcast_to(alphas[c * ND_SH : (c + 1) * ND_SH], (128, ND_SH))
            )
            in_maps.append(
                {"inpT": inpT, "dataT": dataT, "gating": gating, "alphas_b": al}
            )

    res = run_bass_kernel_spmd(nc, in_maps, core_ids=list(range(R * C))).results

    y = np.empty(N_I, dtype=np.float32)
    for r in range(R):
        acc = res[r * C]["y"].T.reshape(NI_SH).copy()
        for c in range(1, C):
            acc += res[r * C + c]["y"].T.reshape(NI_SH)
        y[r * NI_SH : (r + 1) * NI_SH] = acc
    return y
